# revision 1
# baseline (speedup 1.0000x reference)
"""Trainium2 Bass kernel for a 2-branch, 2-layer GCN (nn_Net_7172595384447).

Strategy (8 NeuronCores, SPMD):
  - Nodes sharded across cores by destination (6250 nodes/core, 49 tiles of 128).
  - Self-loops appended as ordinary edges; deg = bincount(dst_aug).
  - Phase A: per-shard dense matmuls h1pre = x@W1, hd1pre = dat@Wd1, scaled by
    dinv -> hs1 table rows; AllGather -> full hs1 table in every core's DRAM.
  - Phase B: per dst tile, dma_gather the incoming edges' source rows, build
    dst one-hot via iota/is_equal, aggregate with PSUM-accumulated matmuls
    (bias folded in as a K=1 "bias edge" whose lhsT row is sqrt(deg)).
    Epilogue produces h1s = dinv*relu(dinv*agg + b); a PE transpose + two
    small matmuls produce hs2 = h1s @ blockdiag(0.2*W2, 0.1*Wd2).
    AllGather -> full hs2 table.
  - Phase C: same aggregation over hs2 rows, then log_softmax per node row.
Host does graph preprocessing only (sharding, sorting by dst, degree counts,
int16 gather-index tables with a -32768 bias so one int16 gather covers all
50176 padded table rows).
"""

import numpy as np

import concourse.bass as bass
import concourse.mybir as mybir
import concourse.tile as tile
from concourse import bacc
from concourse.bass_utils import run_bass_kernel_spmd
from concourse.masks import make_identity

NCORES = 8
N = 50000
FX = 512
FD = 64
SH = N // NCORES            # 6250 nodes per shard
TILES = (SH + 127) // 128   # 49 tiles (48 full + 106)
SH_PAD = TILES * 128        # 6272 padded shard rows
NT = SH_PAD * NCORES        # 50176 padded table rows
BIAS = 32768                # int16 index bias
H1 = 96                     # hs1 used cols (64 + 32)
H1P = 128                   # hs1 padded cols (512B rows)
H2 = 32                     # hs2 used cols (16 + 16)
TDT_BF16 = True             # gather-table / one-hot / msg dtype
H2P = 128 if TDT_BF16 else 64   # hs2 padded cols (256B rows either way)
PAD_DST = 300.0             # dst_local sentinel for pad slots (never matches iota)

_CACHE = {}


def _row_of_node(n):
    return (n // SH) * SH_PAD + (n % SH)


def _host_prep(edge_index):
    src = np.asarray(edge_index[0], dtype=np.int64)
    dst = np.asarray(edge_index[1], dtype=np.int64)
    loops = np.arange(N, dtype=np.int64)
    src_a = np.concatenate([src, loops])
    dst_a = np.concatenate([dst, loops])

    deg = np.bincount(dst_a, minlength=N).astype(np.float64)
    dinv = (1.0 / np.sqrt(deg)).astype(np.float32)
    sqdeg = np.sqrt(deg).astype(np.float32)

    core_of = dst_a // SH
    tile_of = (dst_a % SH) // 128
    dloc_of = (dst_a % SH) % 128

    # group edges by (core, tile)
    edge_rows = _row_of_node(src_a)
    per_ct = [[None] * TILES for _ in range(NCORES)]
    order = np.lexsort((tile_of, core_of))
    so_core = core_of[order]
    so_tile = tile_of[order]
    so_row = edge_rows[order]
    so_dloc = dloc_of[order]
    # boundaries
    key = so_core * TILES + so_tile
    bounds = np.searchsorted(key, np.arange(NCORES * TILES + 1))
    for c in range(NCORES):
        for t in range(TILES):
            k = c * TILES + t
            sl = slice(bounds[k], bounds[k + 1])
            per_ct[c][t] = (so_row[sl], so_dloc[sl])

    # chunks per tile: global max over cores, always >= 1 trailing pad slot
    C_t = [max(len(per_ct[c][t][0]) // 128 + 1 for c in range(NCORES))
           for t in range(TILES)]

    # build per-core gather idx (int16, [128, sum(8*C_t)]) and dst_loc fp32 [128, sum C_t]
    tot_c = sum(C_t)
    idx_all = []
    dloc_all = []
    for c in range(NCORES):
        idx16 = np.zeros((16, tot_c * 8), dtype=np.int16)   # pad slots -> idx 0 (row 32768)
        dloc = np.full((128, tot_c), PAD_DST, dtype=np.float32)
        off = 0
        for t in range(TILES):
            rows, dl = per_ct[c][t]
            ns = C_t[t] * 128
            e = len(rows)
            block = np.zeros(ns, dtype=np.int64)
            block[:e] = rows - BIAS
            block[e:] = 0
            idx16[:, off * 8:(off + C_t[t]) * 8] = (
                block.astype(np.int16).reshape(ns // 16, 16).T)
            dblock = np.full(ns, PAD_DST, dtype=np.float32)
            dblock[:e] = dl.astype(np.float32)
            dloc[:, off:off + C_t[t]] = dblock.reshape(C_t[t], 128).T
            off += C_t[t]
        idx_all.append(np.tile(idx16, (8, 1)).copy())
        dloc_all.append(dloc)

    return dinv, sqdeg, C_t, idx_all, dloc_all


def _build(C_t):
    dt = mybir.dt
    f32 = dt.float32
    tdt = dt.bfloat16 if TDT_BF16 else f32
    tot_c = sum(C_t)
    C_max = max(C_t)

    nc = bacc.Bacc("TRN2", target_bir_lowering=False, debug=False,
                   num_devices=NCORES)
    xT = nc.dram_tensor("xT", [FX, SH], f32, kind="ExternalInput")
    datT = nc.dram_tensor("datT", [FD, SH], f32, kind="ExternalInput")
    W1 = nc.dram_tensor("W1", [FX, 64], f32, kind="ExternalInput")
    Wd1 = nc.dram_tensor("Wd1", [FD, 32], f32, kind="ExternalInput")
    W2s = nc.dram_tensor("W2s", [64, 16], f32, kind="ExternalInput")
    Wd2s = nc.dram_tensor("Wd2s", [32, 16], f32, kind="ExternalInput")
    brow = nc.dram_tensor("brow", [1, H1], f32, kind="ExternalInput")
    bcrow = nc.dram_tensor("bcrow", [1, H2], f32, kind="ExternalInput")
    dinv_t = nc.dram_tensor("dinv_t", [128, TILES], f32, kind="ExternalInput")
    sqdeg_r = nc.dram_tensor("sqdeg_r", [1, SH_PAD], f32, kind="ExternalInput")
    idx_d = nc.dram_tensor("idx", [128, tot_c * 8], dt.int16, kind="ExternalInput")
    dloc_d = nc.dram_tensor("dloc", [128, tot_c], f32, kind="ExternalInput")
    y = nc.dram_tensor("y", [SH, 16], f32, kind="ExternalOutput")

    with tile.TileContext(nc) as tc:
        with tc.tile_pool(name="const", bufs=1) as constp, \
             tc.tile_pool(name="dram", bufs=1, space="DRAM") as dram:
            # resident constants
            idx_t = constp.tile([128, tot_c * 8], dt.int16)
            nc.sync.dma_start(out=idx_t[:], in_=idx_d[:])
            dloc_t = constp.tile([128, tot_c], tdt)
            if TDT_BF16:
                nc.gpsimd.dma_start(out=dloc_t[:], in_=dloc_d[:])
            else:
                nc.sync.dma_start(out=dloc_t[:], in_=dloc_d[:])
            dinv_s = constp.tile([128, TILES], f32)
            nc.sync.dma_start(out=dinv_s[:], in_=dinv_t[:])
            sqdeg_s = constp.tile([1, SH_PAD], f32)
            nc.sync.dma_start(out=sqdeg_s[:], in_=sqdeg_r[:])
            brow_s = constp.tile([1, H1], f32)
            nc.sync.dma_start(out=brow_s[:], in_=brow[:])
            bcrow_s = constp.tile([1, H2], f32)
            nc.sync.dma_start(out=bcrow_s[:], in_=bcrow[:])
            w2b_s = constp.tile([H1, 16], f32)
            nc.sync.dma_start(out=w2b_s[0:64, :], in_=W2s[:])
            nc.sync.dma_start(out=w2b_s[64:96, :], in_=Wd2s[:])
            ident = constp.tile([128, 128], f32)
            make_identity(nc, ident[:])
            iota_i = constp.tile([128, 128], dt.int32)
            nc.gpsimd.iota(iota_i[:], pattern=[[1, 128]], base=0,
                           channel_multiplier=0)
            iota_f = constp.tile([128, 128], tdt)
            nc.vector.tensor_copy(iota_f[:], iota_i[:])

            hs1_shard = dram.tile([SH_PAD, H1P], tdt)
            hs1_full = dram.tile([NT, H1P], tdt)
            hs2_shard = dram.tile([SH_PAD, H2P], tdt)
            hs2_full = dram.tile([NT, H2P], tdt)

            # ---------------- Phase A ----------------
            with tc.tile_pool(name="phA", bufs=2) as pa, \
                 tc.tile_pool(name="phA_w", bufs=1) as paw, \
                 tc.tile_pool(name="psA", bufs=2, space="PSUM") as psa:
                w1_s = paw.tile([128, FX // 128, 64], f32)
                for k in range(FX // 128):
                    nc.sync.dma_start(out=w1_s[:, k, :],
                                      in_=W1[k * 128:(k + 1) * 128, :])
                wd1_s = paw.tile([FD, 32], f32)
                nc.sync.dma_start(out=wd1_s[:], in_=Wd1[:])
                xT_s = []
                for k in range(FX // 128):
                    st = paw.tile([128, SH], f32, tag=f"xT{k}")
                    nc.sync.dma_start(out=st[:],
                                      in_=xT[k * 128:(k + 1) * 128, :])
                    xT_s.append(st)
                datT_s = paw.tile([FD, SH], f32)
                nc.sync.dma_start(out=datT_s[:], in_=datT[:])

                for t in range(TILES):
                    nt = min(128, SH - t * 128)
                    ps = psa.tile([128, H1], f32, space="PSUM", tag="psA")
                    for k in range(FX // 128):
                        nc.tensor.matmul(
                            out=ps[:nt, 0:64],
                            lhsT=xT_s[k][:, t * 128:t * 128 + nt],
                            rhs=w1_s[:, k, :],
                            start=(k == 0), stop=(k == FX // 128 - 1))
                    nc.tensor.matmul(
                        out=ps[:nt, 64:96],
                        lhsT=datT_s[:, t * 128:t * 128 + nt],
                        rhs=wd1_s[:],
                        start=True, stop=True)
                    hso = pa.tile([128, H1P], tdt, tag="hs1o")
                    nc.vector.memset(hso[:, H1:], 0.0)
                    nc.vector.tensor_scalar_mul(
                        hso[:nt, :H1], ps[:nt, :], dinv_s[:nt, t:t + 1])
                    nc.sync.dma_start(
                        out=hs1_shard[t * 128:t * 128 + nt, :], in_=hso[:nt, :])

            nc.gpsimd.collective_compute(
                "AllGather", mybir.AluOpType.bypass,
                replica_groups=[list(range(NCORES))],
                ins=[hs1_shard.opt()], outs=[hs1_full.opt()])

            # ---------------- Phase B ----------------
            with tc.tile_pool(name="phB", bufs=4) as pb, \
                 tc.tile_pool(name="phBw", bufs=3) as pbw, \
                 tc.tile_pool(name="psB", bufs=2, space="PSUM") as psb, \
                 tc.tile_pool(name="psBt", bufs=2, space="PSUM") as psbt, \
                 tc.tile_pool(name="psB2", bufs=2, space="PSUM") as psb2:
                off = 0
                for t in range(TILES):
                    ct = C_t[t]
                    nt = min(128, SH - t * 128)
                    ni = ct * 128
                    msg = pb.tile([128, ct, H1P], tdt, tag="msg")
                    nc.gpsimd.dma_gather(
                        out_ap=msg[:], in_ap=hs1_full[BIAS:, :],
                        idxs_ap=idx_t[:, off * 8:(off + ct) * 8],
                        num_idxs=ni, num_idxs_reg=ni, elem_size=H1P,
                        single_packet=False)
                    W = pbw.tile([128, ct, 128], tdt, tag="W")
                    nc.any.tensor_tensor(
                        out=W[:],
                        in0=iota_f[:, None, :].to_broadcast([128, ct, 128]),
                        in1=dloc_t[:, off:off + ct, None].to_broadcast(
                            [128, ct, 128]),
                        op=mybir.AluOpType.is_equal)
                    ps1 = psb.tile([128, H1], f32, space="PSUM", tag="ps1")
                    for j in range(ct):
                        nc.tensor.matmul(
                            out=ps1[:], lhsT=W[:, j, :], rhs=msg[:, j, :H1],
                            start=(j == 0), stop=False)
                    nc.tensor.matmul(
                        out=ps1[:], lhsT=sqdeg_s[:, t * 128:(t + 1) * 128],
                        rhs=brow_s[:], start=False, stop=True)
                    # h1s = dinv * relu(dinv * agg)
                    tmp = pb.tile([128, H1], f32, tag="tmp")
                    nc.vector.tensor_scalar_mul(
                        tmp[:], ps1[:], dinv_s[:, t:t + 1])
                    h1s = pb.tile([128, H1], f32, tag="h1s")
                    nc.vector.tensor_scalar(
                        out=h1s[:], in0=tmp[:], scalar1=0.0,
                        scalar2=dinv_s[:, t:t + 1],
                        op0=mybir.AluOpType.max, op1=mybir.AluOpType.mult)
                    # transpose -> [96, 128]
                    pst = psbt.tile([H1, 128], f32, space="PSUM", tag="pst")
                    nc.tensor.transpose(out=pst[:], in_=h1s[:], identity=ident[:])
                    h1sT = pb.tile([H1, 128], f32, tag="h1sT")
                    nc.vector.tensor_copy(h1sT[:], pst[:])
                    ps2 = psb2.tile([128, H2], f32, space="PSUM", tag="ps2")
                    nc.tensor.matmul(out=ps2[:, 0:16], lhsT=h1sT[0:64, :],
                                     rhs=w2b_s[0:64, :], start=True, stop=True)
                    nc.tensor.matmul(out=ps2[:, 16:32], lhsT=h1sT[64:96, :],
                                     rhs=w2b_s[64:96, :], start=True, stop=True)
                    hs2o = pb.tile([128, H2P], tdt, tag="hs2o")
                    nc.vector.memset(hs2o[:, H2:], 0.0)
                    nc.vector.tensor_copy(hs2o[:nt, :H2], ps2[:nt, :])
                    nc.sync.dma_start(
                        out=hs2_shard[t * 128:t * 128 + nt, :], in_=hs2o[:nt, :])
                    off += ct

            nc.gpsimd.collective_compute(
                "AllGather", mybir.AluOpType.bypass,
                replica_groups=[list(range(NCORES))],
                ins=[hs2_shard.opt()], outs=[hs2_full.opt()])

            # ---------------- Phase C ----------------
            with tc.tile_pool(name="phC", bufs=4) as pc_, \
                 tc.tile_pool(name="phCw", bufs=3) as pcw, \
                 tc.tile_pool(name="psC", bufs=2, space="PSUM") as psc:
                off = 0
                for t in range(TILES):
                    ct = C_t[t]
                    nt = min(128, SH - t * 128)
                    ni = ct * 128
                    msg = pc_.tile([128, ct, H2P], tdt, tag="msg2")
                    nc.gpsimd.dma_gather(
                        out_ap=msg[:], in_ap=hs2_full[BIAS:, :],
                        idxs_ap=idx_t[:, off * 8:(off + ct) * 8],
                        num_idxs=ni, num_idxs_reg=ni, elem_size=H2P,
                        single_packet=False)
                    W = pcw.tile([128, ct, 128], tdt, tag="W2")
                    nc.any.tensor_tensor(
                        out=W[:],
                        in0=iota_f[:, None, :].to_broadcast([128, ct, 128]),
                        in1=dloc_t[:, off:off + ct, None].to_broadcast(
                            [128, ct, 128]),
                        op=mybir.AluOpType.is_equal)
                    ps3 = psc.tile([128, H2], f32, space="PSUM", tag="ps3")
                    for j in range(ct):
                        nc.tensor.matmul(
                            out=ps3[:], lhsT=W[:, j, :], rhs=msg[:, j, :H2],
                            start=(j == 0), stop=False)
                    nc.tensor.matmul(
                        out=ps3[:], lhsT=sqdeg_s[:, t * 128:(t + 1) * 128],
                        rhs=bcrow_s[:], start=False, stop=True)
                    uh = pc_.tile([128, 16], f32, tag="uh")
                    nc.vector.tensor_copy(uh[:], ps3[:, 16:32])
                    u = pc_.tile([128, 16], f32, tag="u")
                    nc.vector.tensor_tensor(
                        out=u[:], in0=ps3[:, 0:16], in1=uh[:],
                        op=mybir.AluOpType.add)
                    z = pc_.tile([128, 16], f32, tag="z")
                    nc.vector.tensor_scalar_mul(z[:], u[:], dinv_s[:, t:t + 1])
                    m = pc_.tile([128, 1], f32, tag="m")
                    nc.vector.reduce_max(m[:], z[:], axis=mybir.AxisListType.X)
                    zs = pc_.tile([128, 16], f32, tag="zs")
                    nc.vector.tensor_scalar(
                        out=zs[:], in0=z[:], scalar1=m[:, :1], scalar2=None,
                        op0=mybir.AluOpType.subtract)
                    ex = pc_.tile([128, 16], f32, tag="ex")
                    nc.scalar.activation(
                        ex[:], zs[:], mybir.ActivationFunctionType.Exp)
                    s = pc_.tile([128, 1], f32, tag="s")
                    nc.vector.reduce_sum(s[:], ex[:], axis=mybir.AxisListType.X)
                    ls = pc_.tile([128, 1], f32, tag="ls")
                    nc.scalar.activation(
                        ls[:], s[:], mybir.ActivationFunctionType.Ln)
                    ot = pc_.tile([128, 16], f32, tag="ot")
                    nc.vector.tensor_scalar(
                        out=ot[:], in0=zs[:], scalar1=ls[:, :1], scalar2=None,
                        op0=mybir.AluOpType.subtract)
                    nc.sync.dma_start(
                        out=y[t * 128:t * 128 + nt, :], in_=ot[:nt, :])
                    off += ct

    nc.compile()
    return nc


def kernel(x, dat, edge_index, W1, b1, W2, b2, Wd1, bd1, Wd2, bd2):
    x = np.asarray(x, dtype=np.float32)
    dat = np.asarray(dat, dtype=np.float32)
    dinv, sqdeg, C_t, idx_all, dloc_all = _host_prep(np.asarray(edge_index))

    key = tuple(C_t)
    if key not in _CACHE:
        _CACHE[key] = _build(C_t)
    nc = _CACHE[key]

    W1f = np.asarray(W1, np.float32)
    Wd1f = np.asarray(Wd1, np.float32)
    W2s = 0.2 * np.asarray(W2, np.float32)
    Wd2s = 0.1 * np.asarray(Wd2, np.float32)
    brow = np.concatenate([np.asarray(b1, np.float32),
                           np.asarray(bd1, np.float32)])[None, :]
    bcrow = np.concatenate([0.2 * np.asarray(b2, np.float32),
                            0.1 * np.asarray(bd2, np.float32)])[None, :]

    in_maps = []
    for c in range(NCORES):
        lo, hi = c * SH, (c + 1) * SH
        dv = np.zeros((128, TILES), np.float32)
        dv_flat = dinv[lo:hi]
        dv[:, :TILES - 1] = dv_flat[:(TILES - 1) * 128].reshape(TILES - 1, 128).T
        rem = SH - (TILES - 1) * 128
        dv[:rem, TILES - 1] = dv_flat[(TILES - 1) * 128:]
        sq = np.zeros((1, SH_PAD), np.float32)
        sq[0, :SH] = sqdeg[lo:hi]
        in_maps.append({
            "xT": np.ascontiguousarray(x[lo:hi].T),
            "datT": np.ascontiguousarray(dat[lo:hi].T),
            "W1": W1f, "Wd1": Wd1f, "W2s": W2s, "Wd2s": Wd2s,
            "brow": brow, "bcrow": bcrow,
            "dinv_t": dv, "sqdeg_r": sq,
            "idx": idx_all[c], "dloc": dloc_all[c],
        })

    res = run_bass_kernel_spmd(nc, in_maps, core_ids=list(range(NCORES)))
    out = np.concatenate([res.results[c]["y"] for c in range(NCORES)], axis=0)
    return out.astype(np.float32)



# revision 10
# speedup vs baseline: 1.3783x; 1.3783x over previous
"""Trainium2 Bass kernel for a 2-branch, 2-layer GCN (nn_Net_7172595384447).

V3 strategy (8 NeuronCores, SPMD). Profiling the dma_gather baseline showed
two serial bottlenecks: Q7 SWDGE descriptor generation (~9.2ns/edge, all
gathers on queue 0 => one Q7 cpu pair) and the on-device one-hot `is_equal`
builds saturating the Vector engine. V3 keeps the slot-major aggregation
structure but:

  - dma_gather descriptor generation is spread across 4 SWDGE queues
    (queue q runs on Q7 cpu pair q, so gathers on different queues overlap;
    measured 3.2x speedup).
  - One-hot aggregation matrices W are prebuilt on the host and streamed
    from DRAM (graph-constant, shared by both aggregation phases); the
    Vector engine only runs small epilogues.
  - Self-loops are not materialized as edges: phase A keeps a node-major
    copy of each shard's table rows in SBUF and the self term is added
    with one vector add per tile.
  - The two output branches are combined before hop-2 aggregation
    (h2 = h1s @ [0.2*W2 | 0.1*Wd2], 16 cols instead of 32).
  - Phase A (x@W1, dat@Wd1) runs in bf16 with per-tile streaming of xT.

Layout: nodes sharded by destination (6250/core, 49 tiles of 128); per-node
tables in DRAM, rows of 128 bf16 (256B), replicated via AllGather between
phases; per-edge messages gathered with dma_gather (256B rows), aggregated
with PSUM-accumulated one-hot matmuls; bias folded in as a K=1 "bias edge"
whose lhsT row is sqrt(deg).
"""

import os
import numpy as np
import ml_dtypes
PHASES = int(os.environ.get('KPHASES', '3'))
KBMODE = int(os.environ.get('KBMODE', '9'))

import concourse.bass as bass
import concourse.mybir as mybir
import concourse.tile as tile
from concourse import bacc
from concourse.bass_utils import run_bass_kernel_spmd
from concourse.masks import make_identity

NCORES = 8
N = 50000
FX = 512
FD = 64
SH = N // NCORES            # 6250 nodes per shard
TILES = (SH + 127) // 128   # 49 tiles
SH_PAD = TILES * 128        # 6272 padded shard rows
NT = SH_PAD * NCORES        # 50176 padded table rows
BIAS = 32768                # int16 index bias
H1 = 96                     # hop-1 used cols (64 + 32)
H1P = 128                   # table row cols (256B rows)
H2 = 16                     # hop-2 used cols (branches combined)
NQ = int(os.environ.get('KNQ', '4'))   # SWDGE queues for gather overlap

_CACHE = {}


def _host_prep(edge_index):
    src = np.asarray(edge_index[0], dtype=np.int64)
    dst = np.asarray(edge_index[1], dtype=np.int64)

    deg = (np.bincount(dst, minlength=N) + 1.0).astype(np.float64)
    dinv = (1.0 / np.sqrt(deg)).astype(np.float32)
    sqdeg = np.sqrt(deg).astype(np.float32)

    rows = (src // SH) * SH_PAD + (src % SH)
    core_of = dst // SH
    tile_of = (dst % SH) // 128
    dloc_of = (dst % SH) % 128

    order = np.lexsort((tile_of, core_of))
    so_core = core_of[order]
    so_tile = tile_of[order]
    so_row = rows[order]
    so_dloc = dloc_of[order]
    key = so_core * TILES + so_tile
    bounds = np.searchsorted(key, np.arange(NCORES * TILES + 1))

    counts = (bounds[1:] - bounds[:-1]).reshape(NCORES, TILES)
    C_t = np.maximum(1, (counts.max(axis=0) + 127) // 128)
    CTOT = int(C_t.sum())

    idx_all = []
    w_all = []
    for c in range(NCORES):
        idx16 = np.zeros((16, CTOT * 8), dtype=np.int16)
        W = np.zeros((128, CTOT * 128), dtype=np.float32)
        off = 0
        for t in range(TILES):
            k = c * TILES + t
            sl = slice(bounds[k], bounds[k + 1])
            r = so_row[sl]
            dl = so_dloc[sl]
            S = C_t[t] * 128
            L = np.zeros(S, dtype=np.int64)
            L[:len(r)] = r - BIAS
            L[len(r):] = 0      # pads: idx 0 (row 32768) — must be >= 0, the
                                # Q7 trims trailing negative idxs (would drop
                                # real edges and skew ring bookkeeping)
            idx16[:, off * 8:(off + C_t[t]) * 8] = (
                L.astype(np.int16).reshape(S // 16, 16).T)
            i = np.arange(len(r))
            W[i % 128, (off + i // 128) * 128 + dl] = 1.0
            off += C_t[t]
        idx_all.append(np.tile(idx16, (8, 1)).copy())
        w_all.append(W.astype(ml_dtypes.bfloat16))

    return dinv, sqdeg, tuple(int(x) for x in C_t), idx_all, w_all


def _build(C_t):
    dt = mybir.dt
    f32 = dt.float32
    bf16 = dt.bfloat16
    CTOT = sum(C_t)

    nc = bacc.Bacc("TRN2", target_bir_lowering=False, debug=False,
                   num_devices=NCORES, num_swdge_queues=NQ)
    xT = nc.dram_tensor("xT", [FX, SH], bf16, kind="ExternalInput")
    datT = nc.dram_tensor("datT", [FD, SH], bf16, kind="ExternalInput")
    W1 = nc.dram_tensor("W1", [FX, 64], bf16, kind="ExternalInput")
    Wd1 = nc.dram_tensor("Wd1", [FD, 32], bf16, kind="ExternalInput")
    W2c = nc.dram_tensor("W2c", [H1, H2], bf16, kind="ExternalInput")
    brow = nc.dram_tensor("brow", [1, H1], f32, kind="ExternalInput")
    bcrow = nc.dram_tensor("bcrow", [1, H2], f32, kind="ExternalInput")
    dinv_t = nc.dram_tensor("dinv_t", [128, TILES], f32, kind="ExternalInput")
    sqdeg_r = nc.dram_tensor("sqdeg_r", [1, SH_PAD], f32, kind="ExternalInput")
    idx_d = nc.dram_tensor("idx", [128, CTOT * 8], dt.int16,
                           kind="ExternalInput")
    W_d = nc.dram_tensor("W", [128, CTOT * 128], bf16, kind="ExternalInput")
    y = nc.dram_tensor("y", [SH, 16], f32, kind="ExternalOutput")

    with tile.TileContext(nc) as tc:
        with tc.tile_pool(name="const", bufs=1) as constp, \
             tc.tile_pool(name="dram", bufs=1, space="DRAM") as dram:
            idx_t = constp.tile([128, CTOT * 8], dt.int16)
            nc.sync.dma_start(out=idx_t[:], in_=idx_d[:])
            dinv_s = constp.tile([128, TILES], f32)
            nc.sync.dma_start(out=dinv_s[:], in_=dinv_t[:])
            sqdeg_s = constp.tile([1, SH_PAD], f32)
            nc.sync.dma_start(out=sqdeg_s[:], in_=sqdeg_r[:])
            brow_s = constp.tile([1, H1], f32)
            nc.sync.dma_start(out=brow_s[:], in_=brow[:])
            bcrow_s = constp.tile([1, H2], f32)
            nc.sync.dma_start(out=bcrow_s[:], in_=bcrow[:])
            w2c_s = constp.tile([H1, H2], bf16)
            nc.sync.dma_start(out=w2c_s[:], in_=W2c[:])
            ident = constp.tile([128, 128], bf16)
            make_identity(nc, ident[:])

            hs1loc = constp.tile([128, TILES, H1], bf16)
            hs2loc = constp.tile([128, TILES, H2], bf16)

            hs1_shard = dram.tile([SH_PAD, H1P], bf16)
            hs1_full = dram.tile([NT, H1P], bf16)
            hs2_shard = dram.tile([SH_PAD, H1P], bf16)
            hs2_full = dram.tile([NT, H1P], bf16)

            # ---------------- Phase A ----------------
            with tc.tile_pool(name="phA_w", bufs=1) as paw, \
                 tc.tile_pool(name="phA_x", bufs=3) as pax, \
                 tc.tile_pool(name="psA", bufs=2, space="PSUM") as psa:
                w1_s = paw.tile([128, FX // 128, 64], bf16)
                for k in range(FX // 128):
                    nc.sync.dma_start(out=w1_s[:, k, :],
                                      in_=W1[k * 128:(k + 1) * 128, :])
                wd1_s = paw.tile([FD, 32], bf16)
                nc.sync.dma_start(out=wd1_s[:], in_=Wd1[:])

                for t in range(TILES):
                    nt = min(128, SH - t * 128)
                    xt_t = pax.tile([128, FX // 128, 128], bf16, tag="xt")
                    for k in range(FX // 128):
                        nc.sync.dma_start(
                            out=xt_t[:, k, :nt],
                            in_=xT[k * 128:(k + 1) * 128,
                                   t * 128:t * 128 + nt])
                    dat_t = pax.tile([FD, 128], bf16, tag="dat")
                    nc.sync.dma_start(
                        out=dat_t[:, :nt],
                        in_=datT[:, t * 128:t * 128 + nt])
                    ps = psa.tile([128, H1], f32, space="PSUM", tag="psA")
                    for k in range(FX // 128):
                        nc.tensor.matmul(
                            out=ps[:nt, 0:64],
                            lhsT=xt_t[:, k, :nt],
                            rhs=w1_s[:, k, :],
                            start=(k == 0), stop=(k == FX // 128 - 1))
                    nc.tensor.matmul(
                        out=ps[:nt, 64:96],
                        lhsT=dat_t[:, :nt],
                        rhs=wd1_s[:],
                        start=True, stop=True)
                    nc.vector.memset(hs1loc[:, t, :], 0.0)
                    nc.vector.tensor_scalar_mul(
                        hs1loc[:nt, t, :], ps[:nt, :], dinv_s[:nt, t:t + 1])
                    nc.sync.dma_start(
                        out=hs1_shard[t * 128:t * 128 + nt, 0:H1],
                        in_=hs1loc[:nt, t, :])

            nc.gpsimd.collective_compute(
                "AllGather", mybir.AluOpType.bypass,
                replica_groups=[list(range(NCORES))],
                ins=[hs1_shard.opt()], outs=[hs1_full.opt()])

            # ---------------- Phase B ----------------
            if PHASES >= 2:
             with tc.tile_pool(name="phB", bufs=3) as pb, \
                 tc.tile_pool(name="phBw", bufs=4) as pbw, \
                 tc.tile_pool(name="phBg", bufs=4) as pbg, \
                 tc.tile_pool(name="psB1", bufs=2, space="PSUM") as psb1, \
                 tc.tile_pool(name="psBh", bufs=2, space="PSUM") as psbh, \
                 tc.tile_pool(name="psB2", bufs=2, space="PSUM") as psb2:
                off = 0
                for t in range(TILES):
                    ct = C_t[t]
                    nt = min(128, SH - t * 128)
                    ni = ct * 128
                    if KBMODE >= 2 or KBMODE == 0:
                        Wt = pbw.tile([128, ct * 128], bf16, tag="W")
                        nc.sync.dma_start(
                            out=Wt[:], in_=W_d[:, off * 128:(off + ct) * 128])
                    msg = pbg.tile([128, ct, H1P], bf16, tag="msg")
                    nc.gpsimd.dma_gather(
                        out_ap=msg[:], in_ap=hs1_full[BIAS:, :],
                        idxs_ap=idx_t[:, off * 8:(off + ct) * 8],
                        num_idxs=ni, num_idxs_reg=ni, elem_size=H1P,
                        single_packet=False, queue_num=(t % NQ))
                    if KBMODE >= 2:
                        ps1 = psb1.tile([128, H1], f32, space="PSUM",
                                        tag="ps1")
                        for j in range(ct):
                            nc.tensor.matmul(
                                out=ps1[:], lhsT=Wt[:, j * 128:(j + 1) * 128],
                                rhs=msg[:, j, :H1],
                                start=(j == 0), stop=False)
                        nc.tensor.matmul(
                            out=ps1[:], lhsT=sqdeg_s[:, t * 128:(t + 1) * 128],
                            rhs=brow_s[:], start=False, stop=True)
                    if KBMODE >= 3:
                        u = pb.tile([128, H1], f32, tag="u")
                        nc.vector.tensor_tensor(
                            out=u[:], in0=ps1[:], in1=hs1loc[:, t, :],
                            op=mybir.AluOpType.add)
                        h1 = pb.tile([128, H1], f32, tag="h1")
                        nc.vector.tensor_scalar(
                            out=h1[:], in0=u[:], scalar1=dinv_s[:, t:t + 1],
                            scalar2=0.0, op0=mybir.AluOpType.mult,
                            op1=mybir.AluOpType.max)
                        h1s = pb.tile([128, H1], bf16, tag="h1s")
                        nc.vector.tensor_scalar_mul(
                            h1s[:], h1[:], dinv_s[:, t:t + 1])
                    if KBMODE >= 4:
                        psh = psbh.tile([H1, 128], bf16, space="PSUM",
                                        tag="psh")
                        nc.tensor.transpose(out=psh[:], in_=h1s[:],
                                            identity=ident[:])
                        h1sT = pb.tile([H1, 128], bf16, tag="h1sT")
                        nc.scalar.copy(out=h1sT[:], in_=psh[:])
                        ps2 = psb2.tile([128, H2], f32, space="PSUM",
                                        tag="ps2")
                        nc.tensor.matmul(out=ps2[:], lhsT=h1sT[:],
                                         rhs=w2c_s[:], start=True, stop=True)
                        nc.vector.tensor_copy(hs2loc[:, t, :], ps2[:])
                        nc.sync.dma_start(
                            out=hs2_shard[t * 128:t * 128 + nt, 0:H2],
                            in_=hs2loc[:nt, t, :])
                    off += ct

            if PHASES >= 2:
             nc.gpsimd.collective_compute(
                "AllGather", mybir.AluOpType.bypass,
                replica_groups=[list(range(NCORES))],
                ins=[hs2_shard.opt()], outs=[hs2_full.opt()])

            # ---------------- Phase C ----------------
            if PHASES >= 3:
             with tc.tile_pool(name="phC", bufs=3) as pc_, \
                 tc.tile_pool(name="phCw", bufs=4) as pcw, \
                 tc.tile_pool(name="phCg", bufs=4) as pcg, \
                 tc.tile_pool(name="psC1", bufs=2, space="PSUM") as psc1:
                off = 0
                for t in range(TILES):
                    ct = C_t[t]
                    nt = min(128, SH - t * 128)
                    ni = ct * 128
                    Wt = pcw.tile([128, ct * 128], bf16, tag="W2")
                    nc.sync.dma_start(
                        out=Wt[:], in_=W_d[:, off * 128:(off + ct) * 128])
                    msg = pcg.tile([128, ct, H1P], bf16, tag="msg2")
                    nc.gpsimd.dma_gather(
                        out_ap=msg[:], in_ap=hs2_full[BIAS:, :],
                        idxs_ap=idx_t[:, off * 8:(off + ct) * 8],
                        num_idxs=ni, num_idxs_reg=ni, elem_size=H1P,
                        single_packet=False, queue_num=(t % NQ))
                    ps3 = psc1.tile([128, H2], f32, space="PSUM", tag="ps3")
                    for j in range(ct):
                        nc.tensor.matmul(
                            out=ps3[:], lhsT=Wt[:, j * 128:(j + 1) * 128],
                            rhs=msg[:, j, :H2],
                            start=(j == 0), stop=False)
                    nc.tensor.matmul(
                        out=ps3[:], lhsT=sqdeg_s[:, t * 128:(t + 1) * 128],
                        rhs=bcrow_s[:], start=False, stop=True)
                    u3 = pc_.tile([128, H2], f32, tag="u3")
                    nc.vector.tensor_tensor(
                        out=u3[:], in0=ps3[:], in1=hs2loc[:, t, :],
                        op=mybir.AluOpType.add)
                    z = pc_.tile([128, H2], f32, tag="z")
                    nc.vector.tensor_scalar_mul(z[:], u3[:],
                                                dinv_s[:, t:t + 1])
                    m = pc_.tile([128, 1], f32, tag="m")
                    nc.vector.reduce_max(m[:], z[:], axis=mybir.AxisListType.X)
                    zs = pc_.tile([128, H2], f32, tag="zs")
                    nc.vector.tensor_scalar(
                        out=zs[:], in0=z[:], scalar1=m[:, :1], scalar2=None,
                        op0=mybir.AluOpType.subtract)
                    ex = pc_.tile([128, H2], f32, tag="ex")
                    nc.scalar.activation(
                        ex[:], zs[:], mybir.ActivationFunctionType.Exp)
                    s = pc_.tile([128, 1], f32, tag="s")
                    nc.vector.reduce_sum(s[:], ex[:], axis=mybir.AxisListType.X)
                    ls = pc_.tile([128, 1], f32, tag="ls")
                    nc.scalar.activation(
                        ls[:], s[:], mybir.ActivationFunctionType.Ln)
                    ot = pc_.tile([128, H2], f32, tag="ot")
                    nc.vector.tensor_scalar(
                        out=ot[:], in0=zs[:], scalar1=ls[:, :1], scalar2=None,
                        op0=mybir.AluOpType.subtract)
                    nc.sync.dma_start(
                        out=y[t * 128:t * 128 + nt, :], in_=ot[:nt, :])
                    off += ct

            if PHASES < 3:
                with tc.tile_pool(name="stub", bufs=1) as stub:
                    zt = stub.tile([128, 16], f32)
                    nc.vector.memset(zt[:], 0.0)
                    for t in range(TILES):
                        nt = min(128, SH - t * 128)
                        nc.sync.dma_start(out=y[t * 128:t * 128 + nt, :],
                                          in_=zt[:nt, :])

    nc.compile()
    return nc


def kernel(x, dat, edge_index, W1, b1, W2, b2, Wd1, bd1, Wd2, bd2):
    x = np.asarray(x, dtype=np.float32)
    dat = np.asarray(dat, dtype=np.float32)
    dinv, sqdeg, C_t, idx_all, w_all = _host_prep(np.asarray(edge_index))

    if C_t not in _CACHE:
        _CACHE[C_t] = _build(list(C_t))
    nc = _CACHE[C_t]

    bf = ml_dtypes.bfloat16
    W1b = np.asarray(W1, np.float32).astype(bf)
    Wd1b = np.asarray(Wd1, np.float32).astype(bf)
    W2c = np.concatenate([0.2 * np.asarray(W2, np.float32),
                          0.1 * np.asarray(Wd2, np.float32)], axis=0).astype(bf)
    brow = np.concatenate([np.asarray(b1, np.float32),
                           np.asarray(bd1, np.float32)])[None, :]
    bcrow = (0.2 * np.asarray(b2, np.float32) +
             0.1 * np.asarray(bd2, np.float32))[None, :]

    in_maps = []
    for c in range(NCORES):
        lo, hi = c * SH, (c + 1) * SH
        dv = np.zeros((128, TILES), np.float32)
        dv_flat = dinv[lo:hi]
        dv[:, :TILES - 1] = dv_flat[:(TILES - 1) * 128].reshape(
            TILES - 1, 128).T
        rem = SH - (TILES - 1) * 128
        dv[:rem, TILES - 1] = dv_flat[(TILES - 1) * 128:]
        sq = np.zeros((1, SH_PAD), np.float32)
        sq[0, :SH] = sqdeg[lo:hi]
        in_maps.append({
            "xT": np.ascontiguousarray(x[lo:hi].T).astype(bf),
            "datT": np.ascontiguousarray(dat[lo:hi].T).astype(bf),
            "W1": W1b, "Wd1": Wd1b, "W2c": W2c,
            "brow": brow, "bcrow": bcrow,
            "dinv_t": dv, "sqdeg_r": sq,
            "idx": idx_all[c], "W": w_all[c],
        })

    res = run_bass_kernel_spmd(nc, in_maps, core_ids=list(range(NCORES)))
    out = np.concatenate([res.results[c]["y"] for c in range(NCORES)], axis=0)
    return out.astype(np.float32)
